# revision 18
# baseline (speedup 1.0000x reference)
"""Trainium2 Bass kernel for nn_Cellsort_Simulator (gnn_message_passing).

Strategy: data-parallel over batch B=8 across 8 NeuronCores (1 image each).

Math (derived from the reference, exact up to fp rounding):
  w_eff = W_enc @ W_fin  (5,), bias = b_enc@W_fin + b_fin, d = w_eff[0]
  out_dyn[n,h,w] = c(h,w) + d*[n==id(h,w)]  with per-pixel constant c.
  softmax over n collapses: p_other = p0, p_id = p1 (global scalars).
  probs ch0[pix,n] = p0' + (p1'-p0')*[n==id]          (p' = p+EPS)
  probs ch1[pix,t] = base[t] + diff*[t==tid(pix)]
     base[t] = (p0*cntT[t]+EPS)/(1+64*EPS)+EPS, diff = (p1-p0)/(1+64*EPS)
     tid(pix) = type_of_id[id(pix)], cntT[t] = #{k: type_of_id[k]==t}
  pred_disc_id = id if d>0 else ( [id==0] if d<0 else 0 )
  adj from per-cell COM distances (<=15), minus medium (argmax mass) + empty.

On device per core:
  pass A: per w-column one-hot [128p,64n] (DVE is_equal) + one accumulating
          matmul -> stats[6,64] = (mass, sum_h, sum_w, cnt_t1..3).
  smalls: COM/adjacency/type_of_id/base/M on [64]-sized tensors.
  pass C: K=2 broadcast matmul -> flat-ids psum; is_equal -> transposed
          one-hot lhsT [128,512] bf16 (row 127 = ones); per 128-pixel tile
          two matmuls (bf16 hi/lo split of the coefficient matrix) emit both
          prob channels straight into PSUM, +1 matmul gathers tid; PSUM ->
          SBUF copy (DVE/ACT alternating) -> DMA out.
"""

import numpy as np
import ml_dtypes

import concourse.bass as bass
import concourse.bacc as bacc
import concourse.tile as tile
import concourse.mybir as mybir
from concourse.bass_utils import run_bass_kernel_spmd

BF16 = ml_dtypes.bfloat16
F32 = np.float32
ALU = mybir.AluOpType
AX = mybir.AxisListType
ACTF = mybir.ActivationFunctionType

B, H, W, N, T = 8, 128, 128, 64, 4
LAST_RESULT = None
EPS = 1e-6
PIX = H * W  # 16384
NT = PIX // H  # 128 flat tiles of 128 pixels


def _split_hi_lo(a_f32):
    hi = a_f32.astype(BF16)
    lo = (a_f32 - hi.astype(F32)).astype(BF16)
    return hi, lo


def _build_consts(d, p0p, p1p, diffv):
    """Host-side constant tensors shared by all cores."""
    c = {}
    c["iota_row64"] = np.broadcast_to(np.arange(N, dtype=F32), (128, N)).astype(BF16)
    # feat interleaved [128, 128*6]: per w-tile cols = [1, h, w, t1, t2, t3]
    feat = np.zeros((128, 128, 6), dtype=F32)
    feat[:, :, 0] = 1.0
    feat[:, :, 1] = np.arange(128, dtype=F32)[:, None]  # h = partition idx
    feat[:, :, 2] = np.arange(128, dtype=F32)[None, :]  # w = tile idx
    c["feat_init"] = feat.reshape(128, 768).astype(BF16)
    sel2 = np.zeros((2, 128), dtype=F32)
    sel2[0, :N] = 1.0
    sel2[1, 127] = 1.0
    c["sel2"] = sel2.astype(BF16)
    c["ones16k"] = np.ones((1, PIX), dtype=BF16)
    iq = np.full((128, 1), -1.0, dtype=F32)
    iq[:N, 0] = np.arange(N, dtype=F32)
    iq[127, 0] = 1.0
    c["iotaQ"] = iq
    c["iota_sq"] = np.broadcast_to(np.arange(N, dtype=F32), (N, N)).copy()
    c["rev63"] = (63.0 - np.arange(N, dtype=F32)).reshape(1, N)
    c["ones_1x64"] = np.ones((1, N), dtype=F32)
    c["ones_1x128"] = np.ones((1, 128), dtype=F32)
    c["one1"] = np.ones((1, 1), dtype=F32)
    c["ones_col64"] = np.ones((N, 1), dtype=F32)
    c["ident2"] = np.eye(2, dtype=F32)
    c["ident128"] = np.eye(128, dtype=F32)
    # rhs init [128,128]: cols 0:64 = ch0 coefficients, cols 64:128 = ch1 (device fills)
    rhs = np.zeros((128, 128), dtype=F32)
    rhs[:N, :N] = np.eye(N, dtype=F32) * (p1p - p0p)
    rhs[127, :N] = p0p
    hi, lo = _split_hi_lo(rhs)
    c["rhs_hi_init"] = hi
    c["rhs_lo_init"] = lo
    return c


def _build_program(d, p0p, p1p, diffv, base_scale, base_bias):
    nc = bacc.Bacc("TRN2", target_bir_lowering=False, debug=False)
    dt = mybir.dt

    ids_d = nc.dram_tensor("ids", [H, W], dt.int32, kind="ExternalInput")
    types_d = nc.dram_tensor("types", [H, W], dt.int32, kind="ExternalInput")
    cst = {}
    for name, shape, dty in [
        ("iota_row64", [128, N], dt.bfloat16),
        ("feat_init", [128, 768], dt.bfloat16),
        ("sel2", [2, 128], dt.bfloat16),
        ("ones16k", [1, PIX], dt.bfloat16),
        ("iotaQ", [128, 1], dt.float32),
        ("iota_sq", [N, N], dt.float32),
        ("rev63", [1, N], dt.float32),
        ("ones_1x64", [1, N], dt.float32),
        ("ones_1x128", [1, 128], dt.float32),
        ("one1", [1, 1], dt.float32),
        ("ones_col64", [N, 1], dt.float32),
        ("ident2", [2, 2], dt.float32),
        ("ident128", [128, 128], dt.float32),
        ("rhs_hi_init", [128, 128], dt.bfloat16),
        ("rhs_lo_init", [128, 128], dt.bfloat16),
    ]:
        cst[name] = nc.dram_tensor(name, shape, dty, kind="ExternalInput")

    probs0_d = nc.dram_tensor("probs0", [PIX, N], dt.float32, kind="ExternalOutput")
    probs1_d = nc.dram_tensor("probs1", [PIX, N], dt.float32, kind="ExternalOutput")
    pdisc0_d = nc.dram_tensor("pdisc0", [H, W], dt.int32, kind="ExternalOutput")
    pdisc1_d = nc.dram_tensor("pdisc1", [H, W], dt.int32, kind="ExternalOutput")
    tyid_d = nc.dram_tensor("tyid", [N], dt.int32, kind="ExternalOutput")
    adj_d = nc.dram_tensor("adj", [N, N], dt.uint8, kind="ExternalOutput")

    with tile.TileContext(nc) as tc:
        with (
            tc.tile_pool(name="singles", bufs=1) as sg,
            tc.tile_pool(name="oh", bufs=4) as ohp,
            tc.tile_pool(name="sm", bufs=2) as smp,
            tc.tile_pool(name="ps_stats", bufs=1, space="PSUM") as ps_stats,
            tc.tile_pool(name="ps_sm", bufs=2, space="PSUM") as ps_sm,
            tc.tile_pool(name="ps_bc", bufs=2, space="PSUM") as ps_bc,
            tc.tile_pool(name="ps_out", bufs=2, space="PSUM") as ps_out,
            tc.tile_pool(name="ps_tid", bufs=1, space="PSUM") as ps_tid,
            tc.tile_pool(name="oneT", bufs=2) as onep,
            tc.tile_pool(name="stage", bufs=3) as stp,
        ):
            # ---- load inputs + consts
            sb = {}
            for name, t in cst.items():
                s = sg.tile(list(t.shape), t.dtype, tag=name)
                nc.sync.dma_start(out=s[:], in_=t[:])
                sb[name] = s
            ids_sb = sg.tile([H, W], dt.int32, tag="ids_sb")
            nc.sync.dma_start(out=ids_sb[:], in_=ids_d[:])
            types_sb = sg.tile([H, W], dt.int32, tag="types_sb")
            nc.sync.dma_start(out=types_sb[:], in_=types_d[:])

            ids_bf = sg.tile([H, W], dt.bfloat16, tag="ids_bf")
            nc.vector.tensor_copy(out=ids_bf[:], in_=ids_sb[:])
            ids_f = sg.tile([H, W], dt.float32, tag="ids_f")
            nc.vector.tensor_copy(out=ids_f[:], in_=ids_sb[:])
            types_bf = sg.tile([H, W], dt.bfloat16, tag="types_bf")
            nc.vector.tensor_copy(out=types_bf[:], in_=types_sb[:])

            # feat_all: fill type one-hot columns (strided dst)
            feat_all = sb["feat_init"]
            featv = feat_all[:].rearrange("p (w c) -> p w c", c=6)
            for j in (1, 2, 3):
                nc.vector.tensor_scalar(
                    out=featv[:, :, 3 + j - 1],
                    in0=types_bf[:],
                    scalar1=float(j),
                    scalar2=None,
                    op0=ALU.is_equal,
                )

            # flat ids row [2, PIX]: row0 = ids (bf16), row1 = ones
            idsflat2 = sg.tile([2, PIX], dt.bfloat16, tag="idsflat2")
            nc.sync.dma_start(out=idsflat2[0:1, :], in_=ids_bf[:])
            nc.sync.dma_start(out=idsflat2[1:2, :], in_=cst["ones16k"][:])

            # ---- pass A: stats[6, 64] accumulated over 128 w-tiles
            stats_ps = ps_stats.tile([6, N], dt.float32, tag="stats")
            for w in range(W):
                oh = ohp.tile([128, N], dt.bfloat16, tag="oh")
                nc.vector.tensor_scalar(
                    out=oh[:],
                    in0=sb["iota_row64"][:],
                    scalar1=ids_f[:, w : w + 1],
                    scalar2=None,
                    op0=ALU.is_equal,
                )
                nc.tensor.matmul(
                    stats_ps[:],
                    featv[:, w, :],
                    oh[:],
                    start=(w == 0),
                    stop=(w == W - 1),
                )
            stats = sg.tile([6, N], dt.float32, tag="stats_sb")
            nc.vector.tensor_copy(out=stats[:], in_=stats_ps[:])
            # flatten rows onto partition 0 (compute ops need base partition 0)
            statsf = sg.tile([1, 6 * N], dt.float32, tag="statsf")
            nc.sync.dma_start(out=statsf[:], in_=stats[:])
            mass_r = statsf[0:1, 0:N]
            sh_r = statsf[0:1, N : 2 * N]
            sw_r = statsf[0:1, 2 * N : 3 * N]
            c1_r = statsf[0:1, 3 * N : 4 * N]
            c2_r = statsf[0:1, 4 * N : 5 * N]
            c3_r = statsf[0:1, 5 * N : 6 * N]

            # ---- smalls: COM + adjacency
            safe = smp.tile([1, N], dt.float32, tag="r1")
            nc.vector.tensor_scalar(
                out=safe[:], in0=mass_r, scalar1=1.0, scalar2=None, op0=ALU.max
            )
            rs = smp.tile([1, N], dt.float32, tag="r2")
            nc.vector.reciprocal(rs[:], safe[:])
            comh1 = sg.tile([1, N], dt.float32, tag="comh1")
            nc.vector.tensor_tensor(out=comh1[:], in0=sh_r, in1=rs[:], op=ALU.mult)
            comw1 = sg.tile([1, N], dt.float32, tag="comw1")
            nc.vector.tensor_tensor(out=comw1[:], in0=sw_r, in1=rs[:], op=ALU.mult)
            cch_ps = ps_sm.tile([N, 1], dt.float32, tag="ps_small")
            nc.tensor.matmul(cch_ps[:], comh1[:], sb["one1"][:], start=True, stop=True)
            cch = sg.tile([N, 1], dt.float32, tag="cch")
            nc.vector.tensor_copy(out=cch[:], in_=cch_ps[:])
            ccw_ps = ps_sm.tile([N, 1], dt.float32, tag="ps_small")
            nc.tensor.matmul(ccw_ps[:], comw1[:], sb["one1"][:], start=True, stop=True)
            ccw = sg.tile([N, 1], dt.float32, tag="ccw")
            nc.vector.tensor_copy(out=ccw[:], in_=ccw_ps[:])
            bc_ps = ps_sm.tile([N, 2 * N], dt.float32, tag="ps_small")
            nc.tensor.matmul(
                bc_ps[:, 0:N], sb["ones_1x64"][:], comh1[:], start=True, stop=True
            )
            nc.tensor.matmul(
                bc_ps[:, N : 2 * N],
                sb["ones_1x64"][:],
                comw1[:],
                start=True,
                stop=True,
            )
            dh = smp.tile([N, N], dt.float32, tag="m1")
            nc.vector.tensor_scalar(
                out=dh[:],
                in0=bc_ps[:, 0:N],
                scalar1=cch[:, 0:1],
                scalar2=None,
                op0=ALU.subtract,
            )
            dw = smp.tile([N, N], dt.float32, tag="m2")
            nc.vector.tensor_scalar(
                out=dw[:],
                in0=bc_ps[:, N : 2 * N],
                scalar1=ccw[:, 0:1],
                scalar2=None,
                op0=ALU.subtract,
            )
            d2h = smp.tile([N, N], dt.float32, tag="m3")
            nc.vector.tensor_tensor(out=d2h[:], in0=dh[:], in1=dh[:], op=ALU.mult)
            d2w = smp.tile([N, N], dt.float32, tag="m4")
            nc.vector.tensor_tensor(out=d2w[:], in0=dw[:], in1=dw[:], op=ALU.mult)
            dist2 = smp.tile([N, N], dt.float32, tag="m1")
            nc.vector.tensor_tensor(out=dist2[:], in0=d2h[:], in1=d2w[:], op=ALU.add)
            adjf = smp.tile([N, N], dt.float32, tag="m2")
            nc.vector.tensor_scalar(
                out=adjf[:], in0=dist2[:], scalar1=225.0, scalar2=None, op0=ALU.is_le
            )
            # medium (first argmax of mass) + empty masks
            key = smp.tile([1, N], dt.float32, tag="r1")
            nc.vector.tensor_scalar(
                out=key[:], in0=mass_r, scalar1=64.0, scalar2=None, op0=ALU.mult
            )
            key2 = smp.tile([1, N], dt.float32, tag="r2")
            nc.vector.tensor_tensor(
                out=key2[:], in0=key[:], in1=sb["rev63"][:], op=ALU.add
            )
            mk = smp.tile([1, 1], dt.float32, tag="r3")
            nc.vector.tensor_reduce(out=mk[:], in_=key2[:], axis=AX.X, op=ALU.max)
            med = smp.tile([1, N], dt.float32, tag="r4")
            nc.vector.tensor_scalar(
                out=med[:],
                in0=key2[:],
                scalar1=mk[0:1, 0:1],
                scalar2=None,
                op0=ALU.is_equal,
            )
            nz = smp.tile([1, N], dt.float32, tag="r1")
            nc.vector.tensor_scalar(
                out=nz[:], in0=mass_r, scalar1=0.0, scalar2=None, op0=ALU.is_gt
            )
            notmed = smp.tile([1, N], dt.float32, tag="r2")
            nc.vector.tensor_scalar(
                out=notmed[:],
                in0=med[:],
                scalar1=-1.0,
                scalar2=1.0,
                op0=ALU.mult,
                op1=ALU.add,
            )
            good = sg.tile([1, N], dt.float32, tag="good")
            nc.vector.tensor_tensor(
                out=good[:], in0=nz[:], in1=notmed[:], op=ALU.mult
            )
            goodb_ps = ps_sm.tile([N, N], dt.float32, tag="ps_small")
            nc.tensor.matmul(
                goodb_ps[:], sb["ones_1x64"][:], good[:], start=True, stop=True
            )
            goodc_ps = ps_sm.tile([N, 1], dt.float32, tag="ps_small")
            nc.tensor.matmul(goodc_ps[:], good[:], sb["one1"][:], start=True, stop=True)
            goodc = smp.tile([N, 1], dt.float32, tag="r3")
            nc.vector.tensor_copy(out=goodc[:], in_=goodc_ps[:])
            adj2 = smp.tile([N, N], dt.float32, tag="m3")
            nc.vector.tensor_tensor(
                out=adj2[:], in0=adjf[:], in1=goodb_ps[:], op=ALU.mult
            )
            adj_u8 = smp.tile([N, N], dt.uint8, tag="m4")
            nc.vector.tensor_scalar(
                out=adj_u8[:],
                in0=adj2[:],
                scalar1=goodc[:, 0:1],
                scalar2=None,
                op0=ALU.mult,
            )
            nc.sync.dma_start(out=adj_d[:], in_=adj_u8[:])

            # ---- type_of_id
            g1 = smp.tile([1, N], dt.float32, tag="r1")
            nc.vector.tensor_scalar(
                out=g1[:], in0=c1_r, scalar1=0.0, scalar2=1.0,
                op0=ALU.is_gt, op1=ALU.mult,
            )
            g2 = smp.tile([1, N], dt.float32, tag="r2")
            nc.vector.tensor_scalar(
                out=g2[:], in0=c2_r, scalar1=0.0, scalar2=2.0,
                op0=ALU.is_gt, op1=ALU.mult,
            )
            g3 = smp.tile([1, N], dt.float32, tag="r3")
            nc.vector.tensor_scalar(
                out=g3[:], in0=c3_r, scalar1=0.0, scalar2=3.0,
                op0=ALU.is_gt, op1=ALU.mult,
            )
            ty12 = smp.tile([1, N], dt.float32, tag="r4")
            nc.vector.tensor_tensor(out=ty12[:], in0=g1[:], in1=g2[:], op=ALU.max)
            tyr = sg.tile([1, N], dt.float32, tag="tyr")
            nc.vector.tensor_tensor(out=tyr[:], in0=ty12[:], in1=g3[:], op=ALU.max)
            tyi = smp.tile([1, N], dt.int32, tag="r1")
            nc.vector.tensor_copy(out=tyi[:], in_=tyr[:])
            nc.sync.dma_start(out=tyid_d[:], in_=tyi[:])

            # M matrix + base row -> rhs matrices
            tidc_ps = ps_sm.tile([N, 1], dt.float32, tag="ps_small")
            nc.tensor.matmul(tidc_ps[:], tyr[:], sb["one1"][:], start=True, stop=True)
            tidc = sg.tile([N, 1], dt.float32, tag="tidc")
            nc.vector.tensor_copy(out=tidc[:], in_=tidc_ps[:])
            Mf = sg.tile([N, N], dt.float32, tag="Mf")
            nc.vector.tensor_scalar(
                out=Mf[:],
                in0=sb["iota_sq"][:],
                scalar1=tidc[:, 0:1],
                scalar2=None,
                op0=ALU.is_equal,
            )
            cnt_ps = ps_sm.tile([1, N], dt.float32, tag="ps_small")
            nc.tensor.matmul(cnt_ps[:], sb["ones_col64"][:], Mf[:], start=True, stop=True)
            basev = sg.tile([1, N], dt.float32, tag="basev")
            nc.vector.tensor_scalar(
                out=basev[:],
                in0=cnt_ps[:],
                scalar1=float(base_scale),
                scalar2=float(base_bias),
                op0=ALU.mult,
                op1=ALU.add,
            )
            rhs_hi = sb["rhs_hi_init"]
            rhs_lo = sb["rhs_lo_init"]
            Msd = sg.tile([N, N], dt.float32, tag="Msd")
            nc.vector.tensor_scalar(
                out=Msd[:], in0=Mf[:], scalar1=float(diffv), scalar2=None, op0=ALU.mult
            )
            nc.vector.tensor_copy(out=rhs_hi[0:N, N : 2 * N], in_=Msd[:])
            nc.vector.tensor_tensor(
                out=rhs_lo[0:N, N : 2 * N],
                in0=Msd[:],
                in1=rhs_hi[0:N, N : 2 * N],
                op=ALU.subtract,
            )
            base_hi = sg.tile([1, N], dt.bfloat16, tag="base_hi")
            nc.vector.tensor_copy(out=base_hi[:], in_=basev[:])
            base_lo = sg.tile([1, N], dt.bfloat16, tag="base_lo")
            nc.vector.tensor_tensor(
                out=base_lo[:], in0=basev[:], in1=base_hi[:], op=ALU.subtract
            )
            nc.sync.dma_start(out=rhs_hi[127:128, N : 2 * N], in_=base_hi[:])
            nc.sync.dma_start(out=rhs_lo[127:128, N : 2 * N], in_=base_lo[:])
            rhs_tid = sg.tile([128, 1], dt.bfloat16, tag="rhs_tid")
            nc.vector.memset(rhs_tid[:], 0.0)
            nc.vector.tensor_copy(out=rhs_tid[0:N, :], in_=tidc[:])

            # ---- pass C
            tid_ps = ps_tid.tile([128, NT], dt.float32, tag="tid")
            pv0 = probs0_d[:].rearrange("(t p) c -> p t c", p=128)
            pv1 = probs1_d[:].rearrange("(t p) c -> p t c", p=128)
            for g in range(NT // 4):
                bc = ps_bc.tile([128, 512], dt.float32, tag="bc")
                nc.tensor.matmul(
                    bc[:],
                    sb["sel2"][:],
                    idsflat2[:, 512 * g : 512 * (g + 1)],
                    start=True,
                    stop=True,
                )
                oneT = onep.tile([128, 512], dt.bfloat16, tag="oneT")
                nc.vector.tensor_scalar(
                    out=oneT[:],
                    in0=bc[:],
                    scalar1=sb["iotaQ"][:, 0:1],
                    scalar2=None,
                    op0=ALU.is_equal,
                )
                outp = ps_out.tile([128, 512], dt.float32, tag="outp")
                for j in range(4):
                    t = 4 * g + j
                    lh = oneT[:, 128 * j : 128 * (j + 1)]
                    nc.tensor.matmul(
                        outp[:, 128 * j : 128 * (j + 1)], lh, rhs_hi[:],
                        start=True, stop=False,
                    )
                    nc.tensor.matmul(
                        outp[:, 128 * j : 128 * (j + 1)], lh, rhs_lo[:],
                        start=False, stop=True,
                    )
                    nc.tensor.matmul(
                        tid_ps[:, t : t + 1], lh, rhs_tid[:], start=True, stop=True
                    )
                stage = stp.tile([128, 512], dt.float32, tag="stage")
                if g % 2 == 0:
                    nc.vector.tensor_copy(out=stage[:], in_=outp[:])
                else:
                    nc.scalar.activation(stage[:], outp[:], ACTF.Copy)
                sv = stage[:].rearrange("p (j x) -> p j x", j=4)
                nc.sync.dma_start(out=pv0[:, 4 * g : 4 * g + 4, :], in_=sv[:, :, 0:N])
                nc.sync.dma_start(
                    out=pv1[:, 4 * g : 4 * g + 4, :], in_=sv[:, :, N : 2 * N]
                )

            # ---- pred_disc outputs
            if d > 0:
                nc.sync.dma_start(out=pdisc0_d[:], in_=ids_sb[:])
                tid_sb = sg.tile([128, NT], dt.float32, tag="tid_sb")
                nc.vector.tensor_copy(out=tid_sb[:], in_=tid_ps[:])
                tidT_ps = ps_tid.tile([128, 128], dt.float32, tag="tid")
                nc.tensor.transpose(tidT_ps[:], tid_sb[:], sb["ident128"][:])
                tdisc_i = sg.tile([128, 128], dt.int32, tag="tdisc")
                nc.vector.tensor_copy(out=tdisc_i[:], in_=tidT_ps[:])
                nc.sync.dma_start(out=pdisc1_d[:], in_=tdisc_i[:])
            else:
                zval = 0.0 if d < 0 else -1.0  # d==0 -> pmap all zero
                pmap_i = sg.tile([128, 128], dt.int32, tag="pmap_i")
                nc.vector.tensor_scalar(
                    out=pmap_i[:], in0=ids_f[:], scalar1=zval, scalar2=None,
                    op0=ALU.is_equal,
                )
                nc.sync.dma_start(out=pdisc0_d[:], in_=pmap_i[:])
                pmap_f = sg.tile([128, 128], dt.float32, tag="pmap_f")
                nc.vector.tensor_scalar(
                    out=pmap_f[:], in0=ids_f[:], scalar1=zval, scalar2=None,
                    op0=ALU.is_equal,
                )
                ty01_ps = ps_sm.tile([128, 2], dt.float32, tag="ps_small")
                nc.tensor.matmul(
                    ty01_ps[:], sb["ones_1x128"][:], tyr[0:1, 0:2], start=True, stop=True
                )
                ty01 = sg.tile([128, 2], dt.float32, tag="ty01")
                nc.vector.tensor_copy(out=ty01[:], in_=ty01_ps[:])
                dty = sg.tile([128, 1], dt.float32, tag="dty")
                nc.vector.tensor_tensor(
                    out=dty[:], in0=ty01[:, 1:2], in1=ty01[:, 0:1], op=ALU.subtract
                )
                tdisc_i = sg.tile([128, 128], dt.int32, tag="tdisc")
                nc.vector.tensor_scalar(
                    out=tdisc_i[:],
                    in0=pmap_f[:],
                    scalar1=dty[:, 0:1],
                    scalar2=ty01[:, 0:1],
                    op0=ALU.mult,
                    op1=ALU.add,
                )
                nc.sync.dma_start(out=pdisc1_d[:], in_=tdisc_i[:])

    nc.compile()
    return nc


def kernel(cell_ids, cell_types, W_enc, b_enc, W_fin, b_fin):
    cell_ids = np.asarray(cell_ids)
    cell_types = np.asarray(cell_types)
    # host scalar math (float64)
    w_eff = np.asarray(W_enc, np.float64) @ np.asarray(W_fin, np.float64)
    d = float(w_eff[0])
    # softmax over N slots where one slot is d above the rest
    z = np.exp(-abs(d))
    if d >= 0:
        p1 = 1.0 / (1.0 + (N - 1) * z)
        p0 = z / (1.0 + (N - 1) * z)
    else:
        p1 = z / ((N - 1) + z)
        p0 = 1.0 / ((N - 1) + z)
    p0p, p1p = p0 + EPS, p1 + EPS
    den = 1.0 + N * EPS
    diffv = (p1 - p0) / den
    base_scale = p0 / den
    base_bias = EPS / den + EPS

    nc = _build_program(d, p0p, p1p, diffv, base_scale, base_bias)
    consts = _build_consts(d, p0p, p1p, diffv)

    in_maps = []
    for b in range(B):
        m = dict(consts)
        m["ids"] = np.ascontiguousarray(cell_ids[b, 0])
        m["types"] = np.ascontiguousarray(cell_types[b, 0])
        in_maps.append(m)

    res = run_bass_kernel_spmd(nc, in_maps, core_ids=list(range(B)))
    global LAST_RESULT
    LAST_RESULT = res
    results = res.results

    probs = np.stack(
        [
            np.stack(
                [
                    r["probs0"].reshape(H, W, N),
                    r["probs1"].reshape(H, W, N),
                ]
            )
            for r in results
        ]
    ).astype(F32)
    pred_disc = np.stack(
        [np.stack([r["pdisc0"], r["pdisc1"]]) for r in results]
    ).astype(np.int32)
    tyid = np.stack([r["tyid"] for r in results]).astype(np.int32)
    adj = np.stack([r["adj"] for r in results]).astype(bool)
    return probs, pred_disc, tyid, adj
